# revision 21
# baseline (speedup 1.0000x reference)
"""Category-specific 2-layer MLP (MoE-style routing), expert-parallel on 8 NeuronCores.

Math (per sample b with category c = cat_ids[b]):
    h   = relu(x_flat[b] @ W1[c] + b1[c])      x_flat: [32, 4096], W1: [8, 4096, 1024]
    out = h @ W2[c] + b2[c]                    W2: [8, 1024, 512]

Sharding: expert-parallel. Core k holds ONLY category k's weights (16 MB W1 + 2 MB W2)
and computes the full dense MLP for all 32 samples; the host then gathers row b from
core cat_ids[b]. Per-core HBM traffic is ~18.6 MB (the minimum possible when all 8
categories are in use), vs 144 MB for weight replication.

Kernel layout per core (fp32 matmuls stream the MOVING operand at 4 cycles/row, so
keep the moving dim small: stream x^T / h^T at N=32, keep the big weights stationary):
  layer 1: hT[u] [128, 32] (u = 0..7 mid-tiles, one PSUM bank each) accumulated over
           32 K-tiles: lhsT (stationary) = W1[128t:128t+128, 128u:128u+128],
           rhs (moving) = x^T tile [128, 32]. Produces h already transposed for
           layer 2 — no on-chip transpose stage at all.
  bias+relu: ONE DVE scalar_tensor_tensor per mid-tile:
           ht_sb = max(hT_psum + b1T[:, u], 0)   (b1 transposed is per-PARTITION).
  layer 2: transposed too: oT[v] [128, 32] (v = 0..3) over 8 K-tiles:
           lhsT = W2[128u:128u+128, 128v:128v+128], rhs = hT[u] [128, 32];
           evict fuses the b2 add. Output leaves the chip as out^T [512, 32];
           the host gather undoes the transpose for free.
  W1 streams as 8 uneven DMAs ([5,5,5,5,4,4,3,1] K-tiles) — big slabs amortize
  per-DMA latency, the tiny last slab shortens the post-stream PE tail.

Toolchain constraint: this walrus build allows at most ONE sync-wait command per
instruction. The program is structured so every instruction acquires at most one
new semaphore:
  - every W1/W2 slab lives in its own SBUF tile (no slot reuse -> DMAs carry no waits);
  - the xt DMA is placed 8 positions before the first W1 slab DMA, so both land on the
    same HWDGE queue and one cumulative wait covers both;
  - a leading DVE "touch" of the bias tile acquires its queue semaphore before the
    fused bias ops (which then wait only on PE);
  - the kernel-tail drain is split into single-wait drains (_patch_tail_drain).
Verified by _assert_wait_budget at build time.
"""

import numpy as np

import concourse.bass as bass
import concourse.mybir as mybir
import concourse.tile_sem_assignment as _tsa
from concourse import tile
from concourse.bass_utils import run_bass_kernel_spmd

NUM_CAT = 8
B = 32
IN_DIM = 4096   # 16 * 256
MID = 1024
OUT = 512       # 16 * 32
P = 128
KT1 = IN_DIM // P    # 32 k-tiles for layer 1
KT2 = MID // P       # 8 mid-tiles (layer-1 out / layer-2 contraction)
NT = OUT // P        # 4 out-tiles
SLAB_SIZES = (5, 5, 5, 5, 4, 4, 3, 1)  # k-tiles per W1 DMA; sum == KT1
F32 = mybir.dt.float32

HWDGE_QUEUES = 8


class _PatchHwdgeQueues:
    """Pin Tile's HWDGE round-robin to n queues during scheduling."""

    def __init__(self, n: int):
        self.n = n

    def __enter__(self):
        self._saved = _tsa.NUM_HWDGE_SEMS
        _tsa.NUM_HWDGE_SEMS = self.n
        return self

    def __exit__(self, *exc):
        _tsa.NUM_HWDGE_SEMS = self._saved
        return False


def _patch_tail_drain():
    """Split Tile's kernel-tail drain (one wait per live proc) into a chain of
    single-wait drains: this walrus build caps sync-wait commands per instruction
    and rejects the stock multi-wait drain."""
    if getattr(tile.TileContext, "_tail_drain_patched", False):
        return
    from concourse.vector_clock import ScopedClock, VectorClock

    def _drain_and_barrier(self, tick_clock, wait_clock):
        gc = tick_clock.global_clock
        n = len(gc)
        for p in range(n):
            if gc[p] <= 0:
                continue
            sub = [0] * n
            sub[p] = gc[p]
            d = self.nc.sync.drain()
            wait_clock.add_sem_waits(d.ins, ScopedClock({None: VectorClock(sub)}))
        self.nc.all_engine_barrier()
        assert self.sems is not None
        popped = self.nc._tile_sem_poison_stack.pop()
        assert popped is self._sem_poison
        self.nc.clear_and_free_semaphores(list(self.sems.allocated().values()))
        self.nc.all_engine_barrier()

    tile.TileContext._drain_and_barrier = _drain_and_barrier
    tile.TileContext._tail_drain_patched = True


_patch_tail_drain()


def _build_nc() -> bass.Bass:
    nc = bass.Bass()

    # xt[p, t, b] = x_flat[b, t*128 + p]: K-major layout so each DMA partition line
    # is one contiguous 4 KB segment.
    xt = nc.dram_tensor("xt", [P, KT1, B], F32, kind="ExternalInput")
    w1 = nc.dram_tensor("w1", [IN_DIM, MID], F32, kind="ExternalInput")
    w2 = nc.dram_tensor("w2", [MID, OUT], F32, kind="ExternalInput")
    # biast[p, 0:8] = b1[128u + p]; [p, 8:12] = b2[128v + p]; [p, 12] = 0.
    biast = nc.dram_tensor("biast", [P, KT2 + NT + 1], F32, kind="ExternalInput")
    out = nc.dram_tensor("out", [OUT, B], F32, kind="ExternalOutput")  # transposed

    with _PatchHwdgeQueues(HWDGE_QUEUES), tile.TileContext(nc) as tc:
        with (
            tc.tile_pool(name="const", bufs=1) as const,
            tc.tile_pool(name="w1p", bufs=1) as w1p,
            tc.tile_pool(name="w2p", bufs=1) as w2p,
            tc.tile_pool(name="work", bufs=1) as work,
            tc.tile_pool(name="psum", bufs=1, space="PSUM") as psum,
        ):
            # DMA issue order fixes HWDGE queue assignment (round-robin mod 8):
            # 0:biast 1:w2_0 2:xt 3-9:w2_1..7 10-17:w1 slabs. xt (pos 2) and
            # w1 slab 0 (pos 10) share a queue -> one cumulative wait covers both
            # for the first matmul. Each queue later carries a W1 slab, so every
            # small input is covered by the slab waits PE already performs.
            biast_sb = const.tile([P, KT2 + NT + 1], F32)
            nc.sync.dma_start(biast_sb[:], biast[:])

            w2_sbs = []
            for u in range(KT2):
                w2_sb = w2p.tile([P, OUT], F32, tag=f"w2_{u}", name=f"w2sb{u}")
                nc.sync.dma_start(w2_sb[:], w2[P * u : P * (u + 1), :])
                w2_sbs.append(w2_sb)
                if u == 0:
                    xt_sb = const.tile([P, KT1, B], F32)
                    nc.sync.dma_start(xt_sb[:], xt[:])

            # W1 as 8 uneven DMAs; w1_sbs[s][:, f, :] is K-tile (slab_start[s] + f).
            w1_sbs = []
            row = 0
            slab_of_ktile = []
            for s, sz in enumerate(SLAB_SIZES):
                w1_sb = w1p.tile([P, sz, MID], F32, tag=f"w1_{s}", name=f"w1sb{s}")
                nc.sync.dma_start(
                    w1_sb[:],
                    w1[row : row + P * sz, :].rearrange("(f p) n -> p f n", p=P),
                )
                w1_sbs.append(w1_sb)
                slab_of_ktile += [(s, f) for f in range(sz)]
                row += P * sz

            # ---- layer 1: hT[u][128, 32] = (x @ W1)^T mid-tiles, 8 PSUM banks ----
            ht_ps = [
                psum.tile([P, B], F32, tag=f"hT_{u}", name=f"htps{u}")
                for u in range(KT2)
            ]
            for t in range(KT1):
                s, f = slab_of_ktile[t]
                for u in range(KT2):
                    nc.tensor.matmul(
                        ht_ps[u][:],
                        w1_sbs[s][:, f, P * u : P * (u + 1)],
                        xt_sb[:, t, :],
                        start=(t == 0),
                        stop=(t == KT1 - 1),
                    )

            # DVE touch: acquire the biast queue semaphore ahead of the fused
            # bias ops so they only ever wait on PE.
            touch_sb = work.tile([P, 1], F32)
            nc.vector.tensor_copy(touch_sb[:], biast_sb[:, 12:13])

            zero_bc = biast_sb[:, 12:13].to_broadcast((P, B))

            # ---- fused bias+relu evict: ht_sb[:,u,:] = max(hT[u] + b1T[:,u], 0) ----
            ht_sb = work.tile([P, KT2, B], F32)
            for u in range(KT2):
                nc.vector.scalar_tensor_tensor(
                    ht_sb[:, u, :],
                    ht_ps[u][:],
                    biast_sb[:, u : u + 1],
                    zero_bc,
                    mybir.AluOpType.add,
                    mybir.AluOpType.max,
                )

            # ---- layer 2 (transposed): oT[v][128, 32] over 8 K-tiles ----
            # oT psum tiles reuse hT_0..3 slots (released once their evict ran).
            ot_ps = [
                psum.tile([P, B], F32, tag=f"hT_{v}", name=f"otps{v}")
                for v in range(NT)
            ]
            for u in range(KT2):
                for v in range(NT):
                    nc.tensor.matmul(
                        ot_ps[v][:],
                        w2_sbs[u][:, P * v : P * (v + 1)],
                        ht_sb[:, u, :],
                        start=(u == 0),
                        stop=(u == KT2 - 1),
                    )

            # ---- fused bias evict: ot_sb[:,v,:] = oT[v] + b2T[:,v] ----
            ot_sb = work.tile([P, NT, B], F32)
            for v in range(NT):
                nc.vector.scalar_tensor_tensor(
                    ot_sb[:, v, :],
                    ot_ps[v][:],
                    biast_sb[:, KT2 + v : KT2 + v + 1],
                    zero_bc,
                    mybir.AluOpType.add,
                    mybir.AluOpType.add,
                )
            # SWDGE path: a fresh DMA proc, so the store carries only the DVE wait
            # (an HWDGE queue would add a self-queue FIFO wait -> 2 waits).
            nc.gpsimd.dma_start(out.rearrange("(v p) b -> p v b", p=P), ot_sb[:])

    _assert_wait_budget(nc)
    return nc


def _assert_wait_budget(nc: bass.Bass, max_waits: int = 1):
    """This walrus build rejects instructions with >1 sync wait; fail fast."""
    bad = []
    for blk in nc.m.functions[0].blocks:
        for inst in blk.instructions:
            if type(inst).__name__ not in (
                "InstMatmult",
                "InstDMACopy",
                "InstDrain",
                "InstTensorCopy",
                "InstTensorScalarPtr",
            ):
                continue
            si = inst.sync_info
            nw = len(si.on_wait) if si is not None else 0
            if nw > max_waits:
                bad.append(
                    (
                        inst.name,
                        type(inst).__name__,
                        [(w.ant_name, w.wait_value) for w in si.on_wait],
                    )
                )
    if bad:
        raise RuntimeError(f"instructions with >{max_waits} sync waits: {bad}")


_NC_CACHE: bass.Bass | None = None


def _get_nc() -> bass.Bass:
    global _NC_CACHE
    if _NC_CACHE is None:
        _NC_CACHE = _build_nc()
    return _NC_CACHE


def _make_in_maps(x, W1, b1, W2, b2):
    x_flat = np.ascontiguousarray(np.asarray(x, dtype=np.float32)).reshape(B, IN_DIM)
    # xt[p, t, b] = x_flat[b, t*128 + p]
    xt = np.ascontiguousarray(x_flat.reshape(B, KT1, P).transpose(2, 1, 0))
    W1 = np.ascontiguousarray(np.asarray(W1, dtype=np.float32))
    W2 = np.ascontiguousarray(np.asarray(W2, dtype=np.float32))
    b1 = np.asarray(b1, dtype=np.float32)
    b2 = np.asarray(b2, dtype=np.float32)
    biast = np.zeros((NUM_CAT, P, KT2 + NT + 1), dtype=np.float32)
    biast[:, :, :KT2] = b1.reshape(NUM_CAT, KT2, P).transpose(0, 2, 1)
    biast[:, :, KT2 : KT2 + NT] = b2.reshape(NUM_CAT, NT, P).transpose(0, 2, 1)
    return [
        {
            "xt": xt,
            "w1": W1[k],
            "w2": W2[k],
            "biast": biast[k],
        }
        for k in range(NUM_CAT)
    ]


def kernel(x, W1, b1, W2, b2, cat_ids) -> np.ndarray:
    nc = _get_nc()
    in_maps = _make_in_maps(x, W1, b1, W2, b2)
    res = run_bass_kernel_spmd(nc, in_maps, list(range(NUM_CAT))).results
    per_cat = np.stack([np.asarray(res[k]["out"]) for k in range(NUM_CAT)])  # [8, OUT, B]
    cat = np.asarray(cat_ids).astype(np.int64).reshape(B)
    sel = per_cat[cat, :, np.arange(B)]  # [B, OUT] (gather undoes the transpose)
    return np.ascontiguousarray(sel.reshape(B, 16, 32).astype(np.float32))


# revision 22
# speedup vs baseline: 1.0060x; 1.0060x over previous
"""Category-specific 2-layer MLP (MoE-style routing), expert-parallel on 8 NeuronCores.

Math (per sample b with category c = cat_ids[b]):
    h   = relu(x_flat[b] @ W1[c] + b1[c])      x_flat: [32, 4096], W1: [8, 4096, 1024]
    out = h @ W2[c] + b2[c]                    W2: [8, 1024, 512]

Sharding: expert-parallel. Core k holds ONLY category k's weights (16 MB W1 + 2 MB W2)
and computes the full dense MLP for all 32 samples; the host then gathers row b from
core cat_ids[b]. Per-core HBM traffic is ~18.6 MB (the minimum possible when all 8
categories are in use), vs 144 MB for weight replication.

Kernel layout per core (fp32 matmuls stream the MOVING operand at 4 cycles/row, so
keep the moving dim small: stream x^T / h^T at N=32, keep the big weights stationary):
  layer 1: hT[u] [128, 32] (u = 0..7 mid-tiles, one PSUM bank each) accumulated over
           32 K-tiles: lhsT (stationary) = W1[128t:128t+128, 128u:128u+128],
           rhs (moving) = x^T tile [128, 32]. Produces h already transposed for
           layer 2 — no on-chip transpose stage at all.
  bias+relu: ONE DVE scalar_tensor_tensor per mid-tile:
           ht_sb = max(hT_psum + b1T[:, u], 0)   (b1 transposed is per-PARTITION).
  layer 2: transposed too: oT[v] [128, 32] (v = 0..3) over 8 K-tiles:
           lhsT = W2[128u:128u+128, 128v:128v+128], rhs = hT[u] [128, 32];
           evict fuses the b2 add. Output leaves the chip as out^T [512, 32];
           the host gather undoes the transpose for free.
  W1 streams as 8 uneven DMAs ([5,5,5,5,4,4,3,1] K-tiles) — big slabs amortize
  per-DMA latency, the tiny last slab shortens the post-stream PE tail.

Toolchain constraint: this walrus build allows at most ONE sync-wait command per
instruction. The program is structured so every instruction acquires at most one
new semaphore:
  - every W1/W2 slab lives in its own SBUF tile (no slot reuse -> DMAs carry no waits);
  - the xt DMA is placed 8 positions before the first W1 slab DMA, so both land on the
    same HWDGE queue and one cumulative wait covers both;
  - a leading DVE "touch" of the bias tile acquires its queue semaphore before the
    fused bias ops (which then wait only on PE);
  - the kernel-tail drain is split into single-wait drains (_patch_tail_drain).
Verified by _assert_wait_budget at build time.
"""

import numpy as np

import concourse.bass as bass
import concourse.mybir as mybir
import concourse.tile_sem_assignment as _tsa
from concourse import tile
from concourse.bass_utils import run_bass_kernel_spmd

NUM_CAT = 8
B = 32
IN_DIM = 4096   # 16 * 256
MID = 1024
OUT = 512       # 16 * 32
P = 128
KT1 = IN_DIM // P    # 32 k-tiles for layer 1
KT2 = MID // P       # 8 mid-tiles (layer-1 out / layer-2 contraction)
NT = OUT // P        # 4 out-tiles
SLAB_SIZES = (5, 5, 5, 5, 4, 4, 3, 1)  # k-tiles per W1 DMA; sum == KT1
F32 = mybir.dt.float32

HWDGE_QUEUES = 4


class _PatchHwdgeQueues:
    """Pin Tile's HWDGE round-robin to n queues during scheduling."""

    def __init__(self, n: int):
        self.n = n

    def __enter__(self):
        self._saved = _tsa.NUM_HWDGE_SEMS
        _tsa.NUM_HWDGE_SEMS = self.n
        return self

    def __exit__(self, *exc):
        _tsa.NUM_HWDGE_SEMS = self._saved
        return False


def _patch_tail_drain():
    """Split Tile's kernel-tail drain (one wait per live proc) into a chain of
    single-wait drains: this walrus build caps sync-wait commands per instruction
    and rejects the stock multi-wait drain."""
    if getattr(tile.TileContext, "_tail_drain_patched", False):
        return
    from concourse.vector_clock import ScopedClock, VectorClock

    def _drain_and_barrier(self, tick_clock, wait_clock):
        gc = tick_clock.global_clock
        n = len(gc)
        for p in range(n):
            if gc[p] <= 0:
                continue
            sub = [0] * n
            sub[p] = gc[p]
            d = self.nc.sync.drain()
            wait_clock.add_sem_waits(d.ins, ScopedClock({None: VectorClock(sub)}))
        self.nc.all_engine_barrier()
        assert self.sems is not None
        popped = self.nc._tile_sem_poison_stack.pop()
        assert popped is self._sem_poison
        self.nc.clear_and_free_semaphores(list(self.sems.allocated().values()))
        self.nc.all_engine_barrier()

    tile.TileContext._drain_and_barrier = _drain_and_barrier
    tile.TileContext._tail_drain_patched = True


_patch_tail_drain()


def _build_nc() -> bass.Bass:
    nc = bass.Bass()

    # xt[p, t, b] = x_flat[b, t*128 + p]: K-major layout so each DMA partition line
    # is one contiguous 4 KB segment.
    xt = nc.dram_tensor("xt", [P, KT1, B], F32, kind="ExternalInput")
    w1 = nc.dram_tensor("w1", [IN_DIM, MID], F32, kind="ExternalInput")
    w2 = nc.dram_tensor("w2", [MID, OUT], F32, kind="ExternalInput")
    # biast[p, 0:8] = b1[128u + p]; [p, 8:12] = b2[128v + p]; [p, 12] = 0.
    biast = nc.dram_tensor("biast", [P, KT2 + NT + 1], F32, kind="ExternalInput")
    out = nc.dram_tensor("out", [OUT, B], F32, kind="ExternalOutput")  # transposed

    with _PatchHwdgeQueues(HWDGE_QUEUES), tile.TileContext(nc) as tc:
        with (
            tc.tile_pool(name="const", bufs=1) as const,
            tc.tile_pool(name="w1p", bufs=1) as w1p,
            tc.tile_pool(name="w2p", bufs=1) as w2p,
            tc.tile_pool(name="work", bufs=1) as work,
            tc.tile_pool(name="psum", bufs=1, space="PSUM") as psum,
        ):
            # DMA issue order fixes HWDGE queue assignment (round-robin mod 8):
            # 0:biast 1:w2_0 2:xt 3-9:w2_1..7 10-17:w1 slabs. xt (pos 2) and
            # w1 slab 0 (pos 10) share a queue -> one cumulative wait covers both
            # for the first matmul. Each queue later carries a W1 slab, so every
            # small input is covered by the slab waits PE already performs.
            biast_sb = const.tile([P, KT2 + NT + 1], F32)
            nc.sync.dma_start(biast_sb[:], biast[:])

            w2_sbs = []
            for u in range(KT2):
                w2_sb = w2p.tile([P, OUT], F32, tag=f"w2_{u}", name=f"w2sb{u}")
                nc.sync.dma_start(w2_sb[:], w2[P * u : P * (u + 1), :])
                w2_sbs.append(w2_sb)
                if u == 0:
                    xt_sb = const.tile([P, KT1, B], F32)
                    nc.sync.dma_start(xt_sb[:], xt[:])

            # W1 as 8 uneven DMAs; w1_sbs[s][:, f, :] is K-tile (slab_start[s] + f).
            w1_sbs = []
            row = 0
            slab_of_ktile = []
            for s, sz in enumerate(SLAB_SIZES):
                w1_sb = w1p.tile([P, sz, MID], F32, tag=f"w1_{s}", name=f"w1sb{s}")
                nc.sync.dma_start(
                    w1_sb[:],
                    w1[row : row + P * sz, :].rearrange("(f p) n -> p f n", p=P),
                )
                w1_sbs.append(w1_sb)
                slab_of_ktile += [(s, f) for f in range(sz)]
                row += P * sz

            # ---- layer 1: hT[u][128, 32] = (x @ W1)^T mid-tiles, 8 PSUM banks ----
            ht_ps = [
                psum.tile([P, B], F32, tag=f"hT_{u}", name=f"htps{u}")
                for u in range(KT2)
            ]
            for t in range(KT1):
                s, f = slab_of_ktile[t]
                for u in range(KT2):
                    nc.tensor.matmul(
                        ht_ps[u][:],
                        w1_sbs[s][:, f, P * u : P * (u + 1)],
                        xt_sb[:, t, :],
                        start=(t == 0),
                        stop=(t == KT1 - 1),
                    )

            # DVE touch: acquire the biast queue semaphore ahead of the fused
            # bias ops so they only ever wait on PE.
            touch_sb = work.tile([P, 1], F32)
            nc.vector.tensor_copy(touch_sb[:], biast_sb[:, 12:13])

            zero_bc = biast_sb[:, 12:13].to_broadcast((P, B))

            # ---- fused bias+relu evict: ht_sb[:,u,:] = max(hT[u] + b1T[:,u], 0) ----
            ht_sb = work.tile([P, KT2, B], F32)
            for u in range(KT2):
                nc.vector.scalar_tensor_tensor(
                    ht_sb[:, u, :],
                    ht_ps[u][:],
                    biast_sb[:, u : u + 1],
                    zero_bc,
                    mybir.AluOpType.add,
                    mybir.AluOpType.max,
                )

            # ---- layer 2 (transposed): oT[v][128, 32] over 8 K-tiles ----
            # oT psum tiles reuse hT_0..3 slots (released once their evict ran).
            ot_ps = [
                psum.tile([P, B], F32, tag=f"hT_{v}", name=f"otps{v}")
                for v in range(NT)
            ]
            for u in range(KT2):
                for v in range(NT):
                    nc.tensor.matmul(
                        ot_ps[v][:],
                        w2_sbs[u][:, P * v : P * (v + 1)],
                        ht_sb[:, u, :],
                        start=(u == 0),
                        stop=(u == KT2 - 1),
                    )

            # ---- fused bias evict: ot_sb[:,v,:] = oT[v] + b2T[:,v] ----
            ot_sb = work.tile([P, NT, B], F32)
            for v in range(NT):
                nc.vector.scalar_tensor_tensor(
                    ot_sb[:, v, :],
                    ot_ps[v][:],
                    biast_sb[:, KT2 + v : KT2 + v + 1],
                    zero_bc,
                    mybir.AluOpType.add,
                    mybir.AluOpType.add,
                )
            # SWDGE path: a fresh DMA proc, so the store carries only the DVE wait
            # (an HWDGE queue would add a self-queue FIFO wait -> 2 waits).
            nc.gpsimd.dma_start(out.rearrange("(v p) b -> p v b", p=P), ot_sb[:])

    _assert_wait_budget(nc)
    return nc


def _assert_wait_budget(nc: bass.Bass, max_waits: int = 1):
    """This walrus build rejects instructions with >1 sync wait; fail fast."""
    bad = []
    for blk in nc.m.functions[0].blocks:
        for inst in blk.instructions:
            if type(inst).__name__ not in (
                "InstMatmult",
                "InstDMACopy",
                "InstDrain",
                "InstTensorCopy",
                "InstTensorScalarPtr",
            ):
                continue
            si = inst.sync_info
            nw = len(si.on_wait) if si is not None else 0
            if nw > max_waits:
                bad.append(
                    (
                        inst.name,
                        type(inst).__name__,
                        [(w.ant_name, w.wait_value) for w in si.on_wait],
                    )
                )
    if bad:
        raise RuntimeError(f"instructions with >{max_waits} sync waits: {bad}")


_NC_CACHE: bass.Bass | None = None


def _get_nc() -> bass.Bass:
    global _NC_CACHE
    if _NC_CACHE is None:
        _NC_CACHE = _build_nc()
    return _NC_CACHE


def _make_in_maps(x, W1, b1, W2, b2):
    x_flat = np.ascontiguousarray(np.asarray(x, dtype=np.float32)).reshape(B, IN_DIM)
    # xt[p, t, b] = x_flat[b, t*128 + p]
    xt = np.ascontiguousarray(x_flat.reshape(B, KT1, P).transpose(2, 1, 0))
    W1 = np.ascontiguousarray(np.asarray(W1, dtype=np.float32))
    W2 = np.ascontiguousarray(np.asarray(W2, dtype=np.float32))
    b1 = np.asarray(b1, dtype=np.float32)
    b2 = np.asarray(b2, dtype=np.float32)
    biast = np.zeros((NUM_CAT, P, KT2 + NT + 1), dtype=np.float32)
    biast[:, :, :KT2] = b1.reshape(NUM_CAT, KT2, P).transpose(0, 2, 1)
    biast[:, :, KT2 : KT2 + NT] = b2.reshape(NUM_CAT, NT, P).transpose(0, 2, 1)
    return [
        {
            "xt": xt,
            "w1": W1[k],
            "w2": W2[k],
            "biast": biast[k],
        }
        for k in range(NUM_CAT)
    ]


def kernel(x, W1, b1, W2, b2, cat_ids) -> np.ndarray:
    nc = _get_nc()
    in_maps = _make_in_maps(x, W1, b1, W2, b2)
    res = run_bass_kernel_spmd(nc, in_maps, list(range(NUM_CAT))).results
    per_cat = np.stack([np.asarray(res[k]["out"]) for k in range(NUM_CAT)])  # [8, OUT, B]
    cat = np.asarray(cat_ids).astype(np.int64).reshape(B)
    sel = per_cat[cat, :, np.arange(B)]  # [B, OUT] (gather undoes the transpose)
    return np.ascontiguousarray(sel.reshape(B, 16, 32).astype(np.float32))


# revision 25
# speedup vs baseline: 1.0061x; 1.0001x over previous
"""Category-specific 2-layer MLP (MoE-style routing), expert-parallel on 8 NeuronCores.

Math (per sample b with category c = cat_ids[b]):
    h   = relu(x_flat[b] @ W1[c] + b1[c])      x_flat: [32, 4096], W1: [8, 4096, 1024]
    out = h @ W2[c] + b2[c]                    W2: [8, 1024, 512]

Sharding: expert-parallel. Core k holds ONLY category k's weights (16 MB W1 + 2 MB W2)
and computes the full dense MLP for all 32 samples; the host then gathers row b from
core cat_ids[b]. Per-core HBM traffic is ~18.6 MB (the minimum possible when all 8
categories are in use), vs 144 MB for weight replication.

Kernel layout per core (fp32 matmuls stream the MOVING operand at 4 cycles/row, so
keep the moving dim small: stream x^T / h^T at N=32, keep the big weights stationary):
  layer 1: hT[u] [128, 32] (u = 0..7 mid-tiles, one PSUM bank each) accumulated over
           32 K-tiles: lhsT (stationary) = W1[128t:128t+128, 128u:128u+128],
           rhs (moving) = x^T tile [128, 32]. Produces h already transposed for
           layer 2 — no on-chip transpose stage at all.
  bias+relu: ONE DVE scalar_tensor_tensor per mid-tile:
           ht_sb = max(hT_psum + b1T[:, u], 0)   (b1 transposed is per-PARTITION).
  layer 2: transposed too: oT[v] [128, 32] (v = 0..3) over 8 K-tiles:
           lhsT = W2[128u:128u+128, 128v:128v+128], rhs = hT[u] [128, 32];
           evict fuses the b2 add. Output leaves the chip as out^T [512, 32];
           the host gather undoes the transpose for free.
  W1 streams as 8 uneven DMAs ([5,5,5,5,4,4,3,1] K-tiles) — big slabs amortize
  per-DMA latency, the tiny last slab shortens the post-stream PE tail.

Toolchain constraint: this walrus build allows at most ONE sync-wait command per
instruction. The program is structured so every instruction acquires at most one
new semaphore:
  - every W1/W2 slab lives in its own SBUF tile (no slot reuse -> DMAs carry no waits);
  - the xt DMA is placed 8 positions before the first W1 slab DMA, so both land on the
    same HWDGE queue and one cumulative wait covers both;
  - a leading DVE "touch" of the bias tile acquires its queue semaphore before the
    fused bias ops (which then wait only on PE);
  - the kernel-tail drain is split into single-wait drains (_patch_tail_drain).
Verified by _assert_wait_budget at build time.
"""

import numpy as np

import concourse.bass as bass
import concourse.mybir as mybir
import concourse.tile_sem_assignment as _tsa
from concourse import tile
from concourse.bass_utils import run_bass_kernel_spmd

NUM_CAT = 8
B = 32
IN_DIM = 4096   # 16 * 256
MID = 1024
OUT = 512       # 16 * 32
P = 128
KT1 = IN_DIM // P    # 32 k-tiles for layer 1
KT2 = MID // P       # 8 mid-tiles (layer-1 out / layer-2 contraction)
NT = OUT // P        # 4 out-tiles
SLAB_SIZES = (8, 8, 8, 4, 2, 1, 1)  # k-tiles per W1 DMA; sum == KT1
F32 = mybir.dt.float32

HWDGE_QUEUES = 4


class _PatchHwdgeQueues:
    """Pin Tile's HWDGE round-robin to n queues during scheduling."""

    def __init__(self, n: int):
        self.n = n

    def __enter__(self):
        self._saved = _tsa.NUM_HWDGE_SEMS
        _tsa.NUM_HWDGE_SEMS = self.n
        return self

    def __exit__(self, *exc):
        _tsa.NUM_HWDGE_SEMS = self._saved
        return False


def _patch_tail_drain():
    """Split Tile's kernel-tail drain (one wait per live proc) into a chain of
    single-wait drains: this walrus build caps sync-wait commands per instruction
    and rejects the stock multi-wait drain."""
    if getattr(tile.TileContext, "_tail_drain_patched", False):
        return
    from concourse.vector_clock import ScopedClock, VectorClock

    def _drain_and_barrier(self, tick_clock, wait_clock):
        gc = tick_clock.global_clock
        n = len(gc)
        for p in range(n):
            if gc[p] <= 0:
                continue
            sub = [0] * n
            sub[p] = gc[p]
            d = self.nc.sync.drain()
            wait_clock.add_sem_waits(d.ins, ScopedClock({None: VectorClock(sub)}))
        self.nc.all_engine_barrier()
        assert self.sems is not None
        popped = self.nc._tile_sem_poison_stack.pop()
        assert popped is self._sem_poison
        self.nc.clear_and_free_semaphores(list(self.sems.allocated().values()))
        self.nc.all_engine_barrier()

    tile.TileContext._drain_and_barrier = _drain_and_barrier
    tile.TileContext._tail_drain_patched = True


_patch_tail_drain()


def _build_nc() -> bass.Bass:
    nc = bass.Bass()

    # xt[p, t, b] = x_flat[b, t*128 + p]: K-major layout so each DMA partition line
    # is one contiguous 4 KB segment.
    xt = nc.dram_tensor("xt", [P, KT1, B], F32, kind="ExternalInput")
    w1 = nc.dram_tensor("w1", [IN_DIM, MID], F32, kind="ExternalInput")
    w2 = nc.dram_tensor("w2", [MID, OUT], F32, kind="ExternalInput")
    # biast[p, 0:8] = b1[128u + p]; [p, 8:12] = b2[128v + p]; [p, 12] = 0.
    biast = nc.dram_tensor("biast", [P, KT2 + NT + 1], F32, kind="ExternalInput")
    out = nc.dram_tensor("out", [OUT, B], F32, kind="ExternalOutput")  # transposed

    with _PatchHwdgeQueues(HWDGE_QUEUES), tile.TileContext(nc) as tc:
        with (
            tc.tile_pool(name="const", bufs=1) as const,
            tc.tile_pool(name="w1p", bufs=1) as w1p,
            tc.tile_pool(name="w2p", bufs=1) as w2p,
            tc.tile_pool(name="work", bufs=1) as work,
            tc.tile_pool(name="psum", bufs=1, space="PSUM") as psum,
        ):
            # DMA issue order fixes HWDGE queue assignment (round-robin mod 4):
            # 0:xt 1:biast 2:w2a 3:w2b 4+:w1 slabs. xt (pos 0) and w1 slab 0
            # (pos 4) share a queue -> one cumulative wait covers both for the
            # first matmul. Each queue later carries a W1 slab, so every small
            # input is covered by the slab waits PE already performs.
            xt_sb = const.tile([P, KT1, B], F32)
            nc.sync.dma_start(xt_sb[:], xt[:])
            biast_sb = const.tile([P, KT2 + NT + 1], F32)
            nc.sync.dma_start(biast_sb[:], biast[:])

            # W2 in two 1 MB DMAs; w2_sbs[h][:, f, :] is K-tile 4h+f.
            w2_sbs = []
            for h in range(2):
                w2_sb = w2p.tile([P, KT2 // 2, OUT], F32, tag=f"w2_{h}", name=f"w2sb{h}")
                nc.sync.dma_start(
                    w2_sb[:],
                    w2[P * (KT2 // 2) * h : P * (KT2 // 2) * (h + 1), :].rearrange(
                        "(f p) n -> p f n", p=P
                    ),
                )
                w2_sbs.append(w2_sb)

            # W1 as 8 uneven DMAs; w1_sbs[s][:, f, :] is K-tile (slab_start[s] + f).
            w1_sbs = []
            row = 0
            slab_of_ktile = []
            for s, sz in enumerate(SLAB_SIZES):
                w1_sb = w1p.tile([P, sz, MID], F32, tag=f"w1_{s}", name=f"w1sb{s}")
                nc.sync.dma_start(
                    w1_sb[:],
                    w1[row : row + P * sz, :].rearrange("(f p) n -> p f n", p=P),
                )
                w1_sbs.append(w1_sb)
                slab_of_ktile += [(s, f) for f in range(sz)]
                row += P * sz

            # ---- layer 1: hT[u][128, 32] = (x @ W1)^T mid-tiles, 8 PSUM banks ----
            ht_ps = [
                psum.tile([P, B], F32, tag=f"hT_{u}", name=f"htps{u}")
                for u in range(KT2)
            ]
            for t in range(KT1):
                s, f = slab_of_ktile[t]
                for u in range(KT2):
                    nc.tensor.matmul(
                        ht_ps[u][:],
                        w1_sbs[s][:, f, P * u : P * (u + 1)],
                        xt_sb[:, t, :],
                        start=(t == 0),
                        stop=(t == KT1 - 1),
                    )

            # DVE touch: acquire the biast queue semaphore ahead of the fused
            # bias ops so they only ever wait on PE.
            touch_sb = work.tile([P, 1], F32)
            nc.vector.tensor_copy(touch_sb[:], biast_sb[:, 12:13])

            zero_bc = biast_sb[:, 12:13].to_broadcast((P, B))

            # ---- fused bias+relu evict: ht_sb[:,u,:] = max(hT[u] + b1T[:,u], 0) ----
            ht_sb = work.tile([P, KT2, B], F32)
            for u in range(KT2):
                nc.vector.scalar_tensor_tensor(
                    ht_sb[:, u, :],
                    ht_ps[u][:],
                    biast_sb[:, u : u + 1],
                    zero_bc,
                    mybir.AluOpType.add,
                    mybir.AluOpType.max,
                )

            # ---- layer 2 (transposed): oT[v][128, 32] over 8 K-tiles ----
            # oT psum tiles reuse hT_0..3 slots (released once their evict ran).
            ot_ps = [
                psum.tile([P, B], F32, tag=f"hT_{v}", name=f"otps{v}")
                for v in range(NT)
            ]
            for u in range(KT2):
                for v in range(NT):
                    nc.tensor.matmul(
                        ot_ps[v][:],
                        w2_sbs[u // 4][:, u % 4, P * v : P * (v + 1)],
                        ht_sb[:, u, :],
                        start=(u == 0),
                        stop=(u == KT2 - 1),
                    )

            # ---- fused bias evict: ot_sb[:,v,:] = oT[v] + b2T[:,v] ----
            ot_sb = work.tile([P, NT, B], F32)
            for v in range(NT):
                nc.vector.scalar_tensor_tensor(
                    ot_sb[:, v, :],
                    ot_ps[v][:],
                    biast_sb[:, KT2 + v : KT2 + v + 1],
                    zero_bc,
                    mybir.AluOpType.add,
                    mybir.AluOpType.add,
                )
            # SWDGE path: a fresh DMA proc, so the store carries only the DVE wait
            # (an HWDGE queue would add a self-queue FIFO wait -> 2 waits).
            nc.gpsimd.dma_start(out.rearrange("(v p) b -> p v b", p=P), ot_sb[:])

    _assert_wait_budget(nc)
    return nc


def _assert_wait_budget(nc: bass.Bass, max_waits: int = 1):
    """This walrus build rejects instructions with >1 sync wait; fail fast."""
    bad = []
    for blk in nc.m.functions[0].blocks:
        for inst in blk.instructions:
            if type(inst).__name__ not in (
                "InstMatmult",
                "InstDMACopy",
                "InstDrain",
                "InstTensorCopy",
                "InstTensorScalarPtr",
            ):
                continue
            si = inst.sync_info
            nw = len(si.on_wait) if si is not None else 0
            if nw > max_waits:
                bad.append(
                    (
                        inst.name,
                        type(inst).__name__,
                        [(w.ant_name, w.wait_value) for w in si.on_wait],
                    )
                )
    if bad:
        raise RuntimeError(f"instructions with >{max_waits} sync waits: {bad}")


_NC_CACHE: bass.Bass | None = None


def _get_nc() -> bass.Bass:
    global _NC_CACHE
    if _NC_CACHE is None:
        _NC_CACHE = _build_nc()
    return _NC_CACHE


def _make_in_maps(x, W1, b1, W2, b2):
    x_flat = np.ascontiguousarray(np.asarray(x, dtype=np.float32)).reshape(B, IN_DIM)
    # xt[p, t, b] = x_flat[b, t*128 + p]
    xt = np.ascontiguousarray(x_flat.reshape(B, KT1, P).transpose(2, 1, 0))
    W1 = np.ascontiguousarray(np.asarray(W1, dtype=np.float32))
    W2 = np.ascontiguousarray(np.asarray(W2, dtype=np.float32))
    b1 = np.asarray(b1, dtype=np.float32)
    b2 = np.asarray(b2, dtype=np.float32)
    biast = np.zeros((NUM_CAT, P, KT2 + NT + 1), dtype=np.float32)
    biast[:, :, :KT2] = b1.reshape(NUM_CAT, KT2, P).transpose(0, 2, 1)
    biast[:, :, KT2 : KT2 + NT] = b2.reshape(NUM_CAT, NT, P).transpose(0, 2, 1)
    return [
        {
            "xt": xt,
            "w1": W1[k],
            "w2": W2[k],
            "biast": biast[k],
        }
        for k in range(NUM_CAT)
    ]


def kernel(x, W1, b1, W2, b2, cat_ids) -> np.ndarray:
    nc = _get_nc()
    in_maps = _make_in_maps(x, W1, b1, W2, b2)
    res = run_bass_kernel_spmd(nc, in_maps, list(range(NUM_CAT))).results
    per_cat = np.stack([np.asarray(res[k]["out"]) for k in range(NUM_CAT)])  # [8, OUT, B]
    cat = np.asarray(cat_ids).astype(np.int64).reshape(B)
    sel = per_cat[cat, :, np.arange(B)]  # [B, OUT] (gather undoes the transpose)
    return np.ascontiguousarray(sel.reshape(B, 16, 32).astype(np.float32))
